# revision 3
# baseline (speedup 1.0000x reference)
"""ChrEmbed (per-chromosome Dense stack) Trainium2 kernel.

Computes out[b, c, :] = x[:, off_c:off_c+n_c] @ Ws[c] + bs[c] for the 22
chromosome blocks, stacked to [512, 22, 256].

Strategy: 8-way expert parallelism with a uniform SPMD program.
  - Host transposes x during sharding (contraction must sit on the SBUF
    partition axis; DMA transpose doesn't exist for fp32, so do it in numpy
    for free while slicing).
  - The ragged chromosomes (1400..8000 features) are chopped into chunks of
    <= KB*128 contraction rows. Chunks are packed into 8 cores x S segment
    slots, zero-padded to exactly KB blocks, so every core runs the exact
    same instruction stream (run_bass_kernel_spmd shares one NEFF).
  - Per segment: stream xT [128, KB*512] and W [128, KB*256] tiles, run
    KB x 4 matmuls (4 batch blocks of 128) accumulating in 4 PSUM tiles,
    evacuate via DVE to SBUF, DMA out [128, 1024].
  - Host sums partial outputs of chunks belonging to the same chromosome
    and adds biases (zero-cost for the zero biases of this module).

Matmul dtype: float32r (full-rate fp32 storage, TF32-class multiplies,
~2e-4 max rel err) by default; set MODE="f32" for bit-conservative 1/4-rate
fp32.
"""

import numpy as np

import concourse.bass as bass
import concourse.mybir as mybir
import concourse.tile as tile
from concourse import bacc
from concourse.bass_utils import run_bass_kernel_spmd

# ---- problem constants (hardcoded; kernel.py must be self-contained) ----
SNP2CHR = [8000, 7800, 6600, 6200, 6000, 5800, 5200, 5000, 4400, 4600, 4500,
           4400, 3400, 3200, 3000, 3000, 2800, 2700, 2200, 2200, 1400, 1600]
OFFSETS = np.concatenate([[0], np.cumsum(SNP2CHR)]).astype(np.int64)
N_CHR = len(SNP2CHR)
UNITS = 256
BATCH = 512
N_CORES = 8

# ---- sharding structure ----
P = 128            # partition dim / contraction block
KB = 12            # contraction blocks per segment
S = 9              # segments per core
JB = BATCH // P    # batch blocks (4)

MODE = "f32r"      # "f32r" (fast, ~2e-4 rel err) or "f32" (exact, 4x slower PE)


def _make_chunks():
    """Chop chromosomes into (chrom, col_start, ncols<=KB*P) chunks and
    assign to (core, segment) slots."""
    chunks = []
    for c, n in enumerate(SNP2CHR):
        start = 0
        while start < n:
            ncols = min(KB * P, n - start)
            chunks.append((c, int(OFFSETS[c]) + start, ncols))
            start += ncols
    assert len(chunks) <= N_CORES * S, (len(chunks), N_CORES * S)
    return chunks


CHUNKS = _make_chunks()

_NC_CACHE = {}


def _build_program(mode):
    if mode in _NC_CACHE:
        return _NC_CACHE[mode]
    dt_mm = mybir.dt.float32r if mode == "f32r" else mybir.dt.float32
    f32 = mybir.dt.float32

    nc = bacc.Bacc("TRN2", target_bir_lowering=False, debug=False)
    xT_d = nc.dram_tensor("xT", (S, P, KB * BATCH), dt_mm, kind="ExternalInput")
    w_d = nc.dram_tensor("w", (S, P, KB * UNITS), dt_mm, kind="ExternalInput")
    out_d = nc.dram_tensor("out", (S, P, JB * UNITS), f32, kind="ExternalOutput")

    with tile.TileContext(nc) as tc:
        with (
            tc.tile_pool(name="xp", bufs=3) as xp,
            tc.tile_pool(name="wp", bufs=3) as wp,
            tc.tile_pool(name="op", bufs=2) as op,
            tc.tile_pool(name="ps", bufs=2, space="PSUM") as ps,
        ):
            for s in range(S):
                xt = xp.tile([P, KB * BATCH], dt_mm, tag="x")
                wt = wp.tile([P, KB * UNITS], dt_mm, tag="w")
                nc.sync.dma_start(xt[:], xT_d[s])
                nc.sync.dma_start(wt[:], w_d[s])
                psums = [
                    ps.tile([P, UNITS], f32, tag=f"ps{j}", name=f"psum{s}_{j}")
                    for j in range(JB)
                ]
                for kb in range(KB):
                    rhs = wt[:, kb * UNITS:(kb + 1) * UNITS]
                    for j in range(JB):
                        lhsT = xt[:, kb * BATCH + j * P: kb * BATCH + (j + 1) * P]
                        nc.tensor.matmul(
                            psums[j][:], lhsT, rhs,
                            start=(kb == 0), stop=(kb == KB - 1),
                        )
                ot = op.tile([P, JB * UNITS], f32, tag="o")
                for j in range(JB):
                    nc.vector.tensor_copy(ot[:, j * UNITS:(j + 1) * UNITS], psums[j][:])
                nc.sync.dma_start(out_d[s], ot[:])

    nc.compile()
    _NC_CACHE[mode] = nc
    return nc


def _shard_inputs(x, Ws):
    """Build per-core xT/w arrays in the [S, P, KB*free] device layout."""
    in_maps = []
    for core in range(N_CORES):
        in_maps.append({
            "xT": np.zeros((S, P, KB * BATCH), np.float32),
            "w": np.zeros((S, P, KB * UNITS), np.float32),
        })
    for slot, (c, col0, ncols) in enumerate(CHUNKS):
        core, s = divmod(slot, S)
        rel0 = col0 - int(OFFSETS[c])
        xbuf = np.zeros((KB * P, BATCH), np.float32)
        xbuf[:ncols] = x[:, col0:col0 + ncols].T
        in_maps[core]["xT"][s] = (
            xbuf.reshape(KB, P, BATCH).swapaxes(0, 1).reshape(P, KB * BATCH)
        )
        wbuf = np.zeros((KB * P, UNITS), np.float32)
        wbuf[:ncols] = Ws[c][rel0:rel0 + ncols]
        in_maps[core]["w"][s] = (
            wbuf.reshape(KB, P, UNITS).swapaxes(0, 1).reshape(P, KB * UNITS)
        )
    return in_maps


def _gather(results, bs):
    out = np.zeros((BATCH, N_CHR, UNITS), np.float32)
    for slot, (c, _col0, _ncols) in enumerate(CHUNKS):
        core, s = divmod(slot, S)
        part = results[core]["out"][s]                      # [P, JB*UNITS]
        part = part.reshape(P, JB, UNITS).swapaxes(0, 1)    # [JB, P, U]
        out[:, c, :] += part.reshape(BATCH, UNITS)
    for c in range(N_CHR):
        b = np.asarray(bs[c], np.float32)
        if b.any():
            out[:, c, :] += b
    return out


def kernel(x, Ws, bs, _run_kwargs=None):
    x = np.asarray(x, np.float32)
    Ws = [np.asarray(w, np.float32) for w in Ws]
    nc = _build_program(MODE)
    in_maps = _shard_inputs(x, Ws)
    res = run_bass_kernel_spmd(
        nc, in_maps, core_ids=list(range(N_CORES)), **(_run_kwargs or {})
    )
    out = _gather(res.results, bs)
    if _run_kwargs:
        kernel.last_result = res
    return out


# revision 4
# speedup vs baseline: 1.0776x; 1.0776x over previous
"""ChrEmbed (per-chromosome Dense stack) Trainium2 kernel.

Computes out[b, c, :] = x[:, off_c:off_c+n_c] @ Ws[c] + bs[c] for the 22
chromosome blocks, stacked to [512, 22, 256].

Strategy: 8-way expert parallelism with a uniform SPMD program (one NEFF on
all 8 NeuronCores, per-core data).
  - Host transposes x during sharding (contraction must sit on the SBUF
    partition axis; fp32 DMA-transpose doesn't exist, numpy does it free).
  - The ragged chromosomes (11..63 blocks of 128 features) are chopped into
    chunks packed into 8 cores x 8 slots with per-slot capacities
    PROFILE = [16,16,16,14,13,9,7,4] blocks (95 blocks/core, 1.9% padding).
    Every core runs the same instruction stream on its own chunk data.
  - Per slot: stream xT [128, L*512] and W [128, L*256] tiles; per
    contraction block kb: 2 matmuls with W[128,128] halves STATIONARY and
    xT [128, 512] MOVING (N=512 amortizes the expensive 4-byte LDWEIGHTS),
    accumulating into 2 PSUM banks [128u, 512b]; DVE-evacuate, DMA out.
  - Outputs land u-major [u, b]; the host un-transposes during gather and
    sums partial chunks of the same chromosome, then adds biases.

Matmul dtype: float32r (full-rate fp32 storage, TF32-class multiplies,
~2e-4 scale-relative err) by default; MODE="f32" for bit-conservative
1/4-rate fp32.
"""

import numpy as np

import concourse.bass as bass
import concourse.mybir as mybir
import concourse.tile as tile
from concourse import bacc
from concourse.bass_utils import run_bass_kernel_spmd

# ---- problem constants (hardcoded; kernel.py must be self-contained) ----
SNP2CHR = [8000, 7800, 6600, 6200, 6000, 5800, 5200, 5000, 4400, 4600, 4500,
           4400, 3400, 3200, 3000, 3000, 2800, 2700, 2200, 2200, 1400, 1600]
OFFSETS = np.concatenate([[0], np.cumsum(SNP2CHR)]).astype(np.int64)
N_CHR = len(SNP2CHR)
UNITS = 256
BATCH = 512
N_CORES = 8

P = 128                                  # partition / contraction block
PROFILE = [16, 16, 16, 14, 13, 9, 7, 4]  # per-core slot capacities (blocks)
S = len(PROFILE)
C_BLK = sum(PROFILE)                     # 95 blocks per core
SLOT_OFF = np.concatenate([[0], np.cumsum(PROFILE)]).astype(int)

MODE = "f32r"      # "f32r" (fast, ~2e-4 rel err) or "f32" (exact, 4x slower PE)


def _pack_chunks():
    """Chop chromosomes into chunks fitting the slot-size pool (8 copies of
    PROFILE) and assign each chunk to a (core, slot). Greedy best-fit,
    mirrors the offline profile search. Returns list of
    (chrom, col_start, ncols, core, slot)."""
    from collections import Counter

    pool = Counter()
    for L in PROFILE:
        pool[L] += N_CORES
    # slot instances per size, in (core, slot_idx) order
    slot_ids = {}
    for sz in set(PROFILE):
        ids = []
        for core in range(N_CORES):
            for si, L in enumerate(PROFILE):
                if L == sz:
                    ids.append((core, si))
        slot_ids[sz] = ids

    order = sorted(range(N_CHR), key=lambda c: -SNP2CHR[c])
    used = Counter()
    chunks = []
    for c in order:
        rem_rows = SNP2CHR[c]
        col = int(OFFSETS[c])
        while rem_rows > 0:
            rem_blk = -(-rem_rows // P)
            sizes = sorted((s for s in pool if pool[s] > 0), reverse=True)
            assert sizes, "profile infeasible"
            if rem_blk >= sizes[0]:
                take = sizes[0]
            else:
                cands = [s for s in sizes if s >= rem_blk]
                take = min(cands) if cands else sizes[0]
            pool[take] -= 1
            core, si = slot_ids[take][used[take]]
            used[take] += 1
            ncols = min(take * P, rem_rows)
            chunks.append((c, col, ncols, core, si))
            col += ncols
            rem_rows -= ncols
    return chunks


CHUNKS = _pack_chunks()

_NC_CACHE = {}


def _build_program(mode):
    if mode in _NC_CACHE:
        return _NC_CACHE[mode]
    dt_mm = mybir.dt.float32r if mode == "f32r" else mybir.dt.float32
    f32 = mybir.dt.float32

    nc = bacc.Bacc("TRN2", target_bir_lowering=False, debug=False)
    xT_d = nc.dram_tensor("xT", (P, C_BLK * BATCH), dt_mm, kind="ExternalInput")
    w_d = nc.dram_tensor("w", (P, C_BLK * UNITS), dt_mm, kind="ExternalInput")
    out_d = nc.dram_tensor("out", (S, P, 2 * BATCH), f32, kind="ExternalOutput")

    with tile.TileContext(nc) as tc:
        with (
            tc.tile_pool(name="xp", bufs=3) as xp,
            tc.tile_pool(name="wp", bufs=3) as wp,
            tc.tile_pool(name="op", bufs=2) as op,
            tc.tile_pool(name="ps", bufs=2, space="PSUM") as ps,
        ):
            for s in range(S):
                L = PROFILE[s]
                off = int(SLOT_OFF[s])
                xt = xp.tile([P, L * BATCH], dt_mm, tag="x", name=f"xt{s}")
                wt = wp.tile([P, L * UNITS], dt_mm, tag="w", name=f"wt{s}")
                # First slot: per-block DMAs so the first matmul starts
                # after ~256KB instead of after the whole 4MB tile.
                nsplit = L if s == 0 else (2 if s == 1 else 1)
                bnds = [L * i // nsplit for i in range(nsplit + 1)]
                for a, b in zip(bnds[:-1], bnds[1:]):
                    nc.sync.dma_start(
                        xt[:, a * BATCH:b * BATCH],
                        xT_d[:, (off + a) * BATCH:(off + b) * BATCH],
                    )
                    nc.sync.dma_start(
                        wt[:, a * UNITS:b * UNITS],
                        w_d[:, (off + a) * UNITS:(off + b) * UNITS],
                    )
                psums = [
                    ps.tile([P, BATCH], f32, tag=f"ps{h}", name=f"psum{s}_{h}")
                    for h in range(2)
                ]
                for kb in range(L):
                    rhs = xt[:, kb * BATCH:(kb + 1) * BATCH]
                    for h in range(2):
                        lhsT = wt[:, kb * UNITS + h * P: kb * UNITS + (h + 1) * P]
                        nc.tensor.matmul(
                            psums[h][:], lhsT, rhs,
                            start=(kb == 0), stop=(kb == L - 1),
                        )
                ot = op.tile([P, 2 * BATCH], f32, tag="o", name=f"ot{s}")
                for h in range(2):
                    nc.vector.tensor_copy(
                        ot[:, h * BATCH:(h + 1) * BATCH], psums[h][:]
                    )
                nc.scalar.dma_start(out_d[s], ot[:])

    nc.compile()
    _NC_CACHE[mode] = nc
    return nc


def _shard_inputs(x, Ws):
    """Build per-core xT/w arrays in the [P, C_BLK*free] device layout."""
    in_maps = [
        {
            "xT": np.zeros((P, C_BLK * BATCH), np.float32),
            "w": np.zeros((P, C_BLK * UNITS), np.float32),
        }
        for _ in range(N_CORES)
    ]
    xv = in_maps  # alias
    for (c, col0, ncols, core, si) in CHUNKS:
        L = PROFILE[si]
        off = int(SLOT_OFF[si])
        rel0 = col0 - int(OFFSETS[c])
        xbuf = np.zeros((L * P, BATCH), np.float32)
        xbuf[:ncols] = x[:, col0:col0 + ncols].T
        xv[core]["xT"][:, off * BATCH:(off + L) * BATCH] = (
            xbuf.reshape(L, P, BATCH).swapaxes(0, 1).reshape(P, L * BATCH)
        )
        wbuf = np.zeros((L * P, UNITS), np.float32)
        wbuf[:ncols] = Ws[c][rel0:rel0 + ncols]
        xv[core]["w"][:, off * UNITS:(off + L) * UNITS] = (
            wbuf.reshape(L, P, UNITS).swapaxes(0, 1).reshape(P, L * UNITS)
        )
    return in_maps


def _gather(results, bs):
    out = np.zeros((BATCH, N_CHR, UNITS), np.float32)
    for (c, _col0, _ncols, core, si) in CHUNKS:
        r = results[core]["out"][si]                       # [P, 2*BATCH]
        part = r.reshape(P, 2, BATCH).transpose(1, 0, 2)   # [2, P(u), b]
        out[:, c, :] += part.reshape(2 * P, BATCH).T       # [b, u]
    for c in range(N_CHR):
        b = np.asarray(bs[c], np.float32)
        if b.any():
            out[:, c, :] += b
    return out


def kernel(x, Ws, bs, _run_kwargs=None):
    x = np.asarray(x, np.float32)
    Ws = [np.asarray(w, np.float32) for w in Ws]
    nc = _build_program(MODE)
    in_maps = _shard_inputs(x, Ws)
    res = run_bass_kernel_spmd(
        nc, in_maps, core_ids=list(range(N_CORES)), **(_run_kwargs or {})
    )
    out = _gather(res.results, bs)
    if _run_kwargs:
        kernel.last_result = res
    return out


# revision 14
# speedup vs baseline: 1.9059x; 1.7686x over previous
"""ChrEmbed (per-chromosome Dense stack) Trainium2 kernel.

Computes out[b, c, :] = x[:, off_c:off_c+n_c] @ Ws[c] + bs[c] for the 22
chromosome blocks, stacked to [512, 22, 256].

Strategy: 8-way expert parallelism with a uniform SPMD program (one NEFF on
all 8 NeuronCores, per-core data).
  - Host transposes x during sharding (contraction must sit on the SBUF
    partition axis; fp32 DMA-transpose doesn't exist, numpy does it free).
  - The ragged chromosomes (11..63 blocks of 128 features) are chopped into
    chunks packed into 8 cores x 8 slots with per-slot capacities
    PROFILE = [16,16,16,14,13,9,7,4] blocks (95 blocks/core, 1.9% padding).
    Every core runs the same instruction stream on its own chunk data.
  - Per slot: stream xT and W tiles; per contraction block kb: matmuls with
    W[128,128] halves STATIONARY and xT [128, 512] MOVING (N=512 amortizes
    weight loads), accumulating into 2 PSUM banks [128u, 512b];
    DVE-evacuate, DMA out.
  - Outputs land u-major [u, b]; the host un-transposes during gather and
    sums partial chunks of the same chromosome, then adds biases.

SCHEME selects the matmul precision/byte tradeoff (host casts during shard):
  f32   : exact fp32 matmul (4 cyc/row, PE-bound ~190us)
  f32r  : fp32 storage, TF32-class multiplies (~1.5e-4 scale-rel err)
  f16x3 : x and W both split into fp16 hi+lo, 3 cross terms
          (~1e-6 err, same bytes as f32r, faster PE)
  f16x2 : x split fp16 hi+lo, W single fp16 (~2.5e-4 err, 0.75x bytes)
  f16   : everything single fp16 (~4.5e-4 err, 0.55x bytes, fastest)
"""

import numpy as np

import concourse.bass as bass
import concourse.mybir as mybir
import concourse.tile as tile
from concourse import bacc
from concourse.bass_utils import run_bass_kernel_spmd

# ---- problem constants (hardcoded; kernel.py must be self-contained) ----
SNP2CHR = [8000, 7800, 6600, 6200, 6000, 5800, 5200, 5000, 4400, 4600, 4500,
           4400, 3400, 3200, 3000, 3000, 2800, 2700, 2200, 2200, 1400, 1600]
OFFSETS = np.concatenate([[0], np.cumsum(SNP2CHR)]).astype(np.int64)
N_CHR = len(SNP2CHR)
UNITS = 256
BATCH = 512
N_CORES = 8

P = 128                                  # partition / contraction block
PROFILE = [16, 16, 16, 14, 13, 9, 7, 4]  # per-core slot capacities (blocks)
S = len(PROFILE)
C_BLK = sum(PROFILE)                     # 95 blocks per core
SLOT_OFF = np.concatenate([[0], np.cumsum(PROFILE)]).astype(int)

import os
SCHEME = os.environ.get("CHREMBED_SCHEME", "f16")

_CFG = {
    #        nx  nw  dtype
    "f32":  (1, 1, mybir.dt.float32),
    "f32r": (1, 1, mybir.dt.float32r),
    "f16":  (1, 1, mybir.dt.float16),
    "f16x2": (2, 1, mybir.dt.float16),
    "f16x3": (2, 2, mybir.dt.float16),
}


def _pack_chunks():
    """Chop chromosomes into chunks fitting the slot-size pool (8 copies of
    PROFILE) and assign each chunk to a (core, slot). Greedy best-fit.
    Returns list of (chrom, col_start, ncols, core, slot)."""
    from collections import Counter

    pool = Counter()
    for L in PROFILE:
        pool[L] += N_CORES
    slot_ids = {}
    for sz in set(PROFILE):
        ids = []
        for core in range(N_CORES):
            for si, L in enumerate(PROFILE):
                if L == sz:
                    ids.append((core, si))
        slot_ids[sz] = ids

    order = sorted(range(N_CHR), key=lambda c: -SNP2CHR[c])
    used = Counter()
    chunks = []
    for c in order:
        rem_rows = SNP2CHR[c]
        col = int(OFFSETS[c])
        while rem_rows > 0:
            rem_blk = -(-rem_rows // P)
            sizes = sorted((s for s in pool if pool[s] > 0), reverse=True)
            assert sizes, "profile infeasible"
            if rem_blk >= sizes[0]:
                take = sizes[0]
            else:
                cands = [s for s in sizes if s >= rem_blk]
                take = min(cands) if cands else sizes[0]
            pool[take] -= 1
            core, si = slot_ids[take][used[take]]
            used[take] += 1
            ncols = min(take * P, rem_rows)
            chunks.append((c, col, ncols, core, si))
            col += ncols
            rem_rows -= ncols
    return chunks


CHUNKS = _pack_chunks()

_NC_CACHE = {}


def _build_program(scheme):
    if scheme in _NC_CACHE:
        return _NC_CACHE[scheme]
    nx, nw, dt_mm = _CFG[scheme]
    f32 = mybir.dt.float32
    dt_out = mybir.dt.float16 if scheme == "f16" else f32
    XW = nx * BATCH      # x elements per block (all terms)
    WW = nw * UNITS      # w elements per block

    nc = bacc.Bacc("TRN2", target_bir_lowering=False, debug=False)
    xT_d = nc.dram_tensor("xT", (P, C_BLK * XW), dt_mm, kind="ExternalInput")
    w_d = nc.dram_tensor("w", (P, C_BLK * WW), dt_mm, kind="ExternalInput")
    out_d = nc.dram_tensor("out", (S, P, 2 * BATCH), dt_out, kind="ExternalOutput")

    with tile.TileContext(nc) as tc:
        with (
            tc.tile_pool(name="xp", bufs=3) as xp,
            tc.tile_pool(name="wp", bufs=3) as wp,
            tc.tile_pool(name="op", bufs=2) as op,
            tc.tile_pool(name="ps", bufs=3, space="PSUM") as ps,
        ):
            for s in range(S):
                L = PROFILE[s]
                off = int(SLOT_OFF[s])
                xt = xp.tile([P, L * XW], dt_mm, tag="x", name=f"xt{s}")
                wt = wp.tile([P, L * WW], dt_mm, tag="w", name=f"wt{s}")
                # First slots: split DMAs so the first matmuls start early.
                nsplit = 4 if s == 0 else (2 if s == 1 else 1)
                bnds = [L * i // nsplit for i in range(nsplit + 1)]
                for a, b in zip(bnds[:-1], bnds[1:]):
                    nc.sync.dma_start(
                        xt[:, a * XW:b * XW], xT_d[:, (off + a) * XW:(off + b) * XW]
                    )
                    nc.scalar.dma_start(
                        wt[:, a * WW:b * WW], w_d[:, (off + a) * WW:(off + b) * WW]
                    )
                psums = [
                    ps.tile([P, BATCH], f32, tag=f"ps{h}", name=f"psum{s}_{h}")
                    for h in range(2)
                ]
                # terms: (wi, xi) pairs; skip lo*lo for f16x3
                terms = [(wi, xi) for wi in range(nw) for xi in range(nx)
                         if wi + xi < max(nx, nw)]
                for kb in range(L):
                    first, last = (kb == 0), (kb == L - 1)
                    for h in range(2):
                        for ti, (wi, xi) in enumerate(terms):
                            lhsT = wt[:, kb * WW + wi * UNITS + h * P:
                                      kb * WW + wi * UNITS + (h + 1) * P]
                            rhs = xt[:, kb * XW + xi * BATCH:
                                     kb * XW + (xi + 1) * BATCH]
                            nc.tensor.matmul(
                                psums[h][:], lhsT, rhs,
                                start=first and ti == 0,
                                stop=last and ti == len(terms) - 1,
                            )
                ot = op.tile([P, 2 * BATCH], dt_out, tag="o", name=f"ot{s}")
                for h in range(2):
                    nc.vector.tensor_copy(
                        ot[:, h * BATCH:(h + 1) * BATCH], psums[h][:]
                    )
                    nc.scalar.dma_start(
                        out_d[s, :, h * BATCH:(h + 1) * BATCH],
                        ot[:, h * BATCH:(h + 1) * BATCH],
                    )

    nc.compile()
    _NC_CACHE[scheme] = nc
    return nc


def _split_terms(a, n_terms, np_dt):
    """Represent fp32 array as sum of n_terms arrays of dtype np_dt."""
    if n_terms == 1:
        return [np.ascontiguousarray(a, np_dt)]
    hi = a.astype(np_dt)
    lo = (a - hi.astype(np.float32)).astype(np_dt)
    return [hi, lo]


def _shard_inputs(x, Ws, scheme):
    nx, nw, dt_mm = _CFG[scheme]
    np_dt = mybir.dt.np(dt_mm)
    XW = nx * BATCH
    WW = nw * UNITS
    in_maps = [
        {
            "xT": np.zeros((P, C_BLK * XW), np_dt),
            "w": np.zeros((P, C_BLK * WW), np_dt),
        }
        for _ in range(N_CORES)
    ]
    for (c, col0, ncols, core, si) in CHUNKS:
        L = PROFILE[si]
        off = int(SLOT_OFF[si])
        rel0 = col0 - int(OFFSETS[c])

        xterms = _split_terms(x[:, col0:col0 + ncols].T, nx, np_dt)
        xbuf = np.zeros((L * P, nx, BATCH), np_dt)
        for xi, t in enumerate(xterms):
            xbuf[:ncols, xi, :] = t
        in_maps[core]["xT"][:, off * XW:(off + L) * XW] = (
            xbuf.reshape(L, P, XW).swapaxes(0, 1).reshape(P, L * XW)
        )

        wterms = _split_terms(Ws[c][rel0:rel0 + ncols], nw, np_dt)
        wbuf = np.zeros((L * P, nw, UNITS), np_dt)
        for wi, t in enumerate(wterms):
            wbuf[:ncols, wi, :] = t
        in_maps[core]["w"][:, off * WW:(off + L) * WW] = (
            wbuf.reshape(L, P, WW).swapaxes(0, 1).reshape(P, L * WW)
        )
    return in_maps


def _gather(results, bs):
    out = np.zeros((BATCH, N_CHR, UNITS), np.float32)
    for (c, _col0, _ncols, core, si) in CHUNKS:
        r = results[core]["out"][si]                       # [P, 2*BATCH]
        part = r.reshape(P, 2, BATCH).transpose(1, 0, 2)   # [2, P(u), b]
        out[:, c, :] += part.reshape(2 * P, BATCH).T       # [b, u]
    for c in range(N_CHR):
        b = np.asarray(bs[c], np.float32)
        if b.any():
            out[:, c, :] += b
    return out


def kernel(x, Ws, bs, _run_kwargs=None):
    x = np.asarray(x, np.float32)
    Ws = [np.asarray(w, np.float32) for w in Ws]
    nc = _build_program(SCHEME)
    in_maps = _shard_inputs(x, Ws, SCHEME)
    res = run_bass_kernel_spmd(
        nc, in_maps, core_ids=list(range(N_CORES)), **(_run_kwargs or {})
    )
    out = _gather(res.results, bs)
    if _run_kwargs:
        kernel.last_result = res
    return out
